# revision 12
# baseline (speedup 1.0000x reference)
"""GatedSlotAttention2 Trainium2 Bass kernel (v2).

Sharding: 2 heads per core x 8 cores (H=16); host sums the 8 partial
Wo outputs. Chunked scan with C=128, all heavy matmul operands in bf16,
state-independent work hoisted out of the sequential loop, single
act-table discipline (Exp/Tanh/Square/Copy + two batched Ln regions),
softmax denominator folded into the RMSNorm eps term.
"""
import numpy as np
import ml_dtypes

import concourse.bass as bass
import concourse.bacc as bacc_mod
import concourse.mybir as mybir
import concourse.tile as tile
from concourse.bass_utils import run_bass_kernel_spmd

# Prefer the activation table that holds Exp+Ln+Square+Copy together so the
# act-table placement pass never needs a mid-kernel table switch.
_orig_get_tables = bacc_mod.get_activation_tables
def _pinned_tables(arch):
    # act_func_set_id is positional (index into act_info.json), so keep
    # order/names and instead empty the sets before the preferred table so
    # first-match resolves every used func to it.
    tabs = _orig_get_tables(arch)
    pref = 'natural_log_exp_and_others'
    if pref not in tabs:
        return tabs
    out = {}
    seen_pref = False
    for k, v in tabs.items():
        if k == pref:
            seen_pref = True
            out[k] = v
        else:
            out[k] = v if seen_pref else type(v)()
    return out
bacc_mod.get_activation_tables = _pinned_tables

F32 = mybir.dt.float32
BF16 = mybir.dt.bfloat16
FP16 = mybir.dt.float16
AF = mybir.ActivationFunctionType
ALU = mybir.AluOpType
MS = bass.MemorySpace

B, T, HID = 1, 1024, 2048
H, DK, DV, M, KW = 16, 128, 128, 128, 4
SCALE = DK ** -0.5
EPS = 1e-5
C = 128           # chunk length
HC = C // 2
NCH = T // C      # 8 chunks
NKT = HID // 128  # 16 contraction tiles
HL = 2            # heads per core
NCT = 10          # 128-wide projection column tiles in wbig

_CACHE = {}


def _build_nc():
    nc = bacc_mod.Bacc("TRN2")

    # ---------------- DRAM I/O ----------------
    d_xt = nc.dram_tensor("xt", [128, NKT * T], BF16, kind="ExternalInput")
    d_wbig = nc.dram_tensor("wbig", [NCT, 128, NKT * 128], BF16, kind="ExternalInput")
    d_wb = nc.dram_tensor("wb", [128, NKT * HL], BF16, kind="ExternalInput")
    d_wf2 = nc.dram_tensor("wf2", [DV, HL * M], BF16, kind="ExternalInput")
    d_wg2 = nc.dram_tensor("wg2", [DV, HL * DV], BF16, kind="ExternalInput")
    d_bg2 = nc.dram_tensor("bg2", [1, HL * DV], BF16, kind="ExternalInput")
    d_wo = nc.dram_tensor("wo", [HL * DV, HID], BF16, kind="ExternalInput")
    d_convw = nc.dram_tensor("convw", [128, 8, KW], F32, kind="ExternalInput")
    d_mcum = nc.dram_tensor("mcum", [C, C], F32, kind="ExternalInput")
    d_mcen = nc.dram_tensor("mcen", [C, C], F32, kind="ExternalInput")
    d_mrev = nc.dram_tensor("mrev", [C, C], F32, kind="ExternalInput")
    d_negones = nc.dram_tensor("negones", [C, 128], F32, kind="ExternalInput")
    d_negcol = nc.dram_tensor("negcol", [C, 1], F32, kind="ExternalInput")
    d_trimask = nc.dram_tensor("trimask", [C, C], BF16, kind="ExternalInput")
    d_identb = nc.dram_tensor("identb", [128, 128], BF16, kind="ExternalInput")
    d_ones1 = nc.dram_tensor("ones1", [1, C], BF16, kind="ExternalInput")
    d_out = nc.dram_tensor("out", [T, HID], FP16, kind="ExternalOutput")

    with tile.TileContext(nc) as tc:
        with (
            tc.tile_pool(name="persist", bufs=1) as pp,
            tc.tile_pool(name="scr", bufs=3) as scr,
            tc.tile_pool(name="psA", bufs=2, space=MS.PSUM) as psA,
            tc.tile_pool(name="psB", bufs=2, space=MS.PSUM) as psB,
            tc.tile_pool(name="psC", bufs=4, space=MS.PSUM) as psC,
        ):
            # ---------- constants ----------
            def load_const(dram, shape, dtype=F32):
                t = pp.tile(shape, dtype, tag=dram.name + "_sb")
                nc.sync.dma_start(t[:], dram[:])
                return t

            c_mcum = load_const(d_mcum, [C, C])
            c_mcen = load_const(d_mcen, [C, C])
            c_mrev = load_const(d_mrev, [C, C])
            c_negones = load_const(d_negones, [C, 128])
            c_negcol = load_const(d_negcol, [C, 1])
            c_trimask = load_const(d_trimask, [C, C], BF16)
            c_identb = load_const(d_identb, [128, 128], BF16)
            c_ones1 = load_const(d_ones1, [1, C], BF16)
            c_wf2 = load_const(d_wf2, [DV, HL * M], BF16)
            c_wg2 = load_const(d_wg2, [DV, HL * DV], BF16)
            c_bg2 = load_const(d_bg2, [1, HL * DV], BF16)
            c_convw = load_const(d_convw, [128, 8, KW])
            c_eps6 = pp.tile([C, 1], F32, tag="c_eps6")
            nc.vector.memset(c_eps6[:], 1e-6)

            # ---------- big loads ----------
            xt_sb = pp.tile([128, NKT, T], BF16, tag="xt_sb")
            nc.sync.dma_start(xt_sb[:], d_xt[:])
            wb_sb = pp.tile([128, NKT, HL], BF16, tag="wb_sb")
            nc.sync.dma_start(wb_sb[:], d_wb[:])
            wo_sb = pp.tile([128, HL, HID], BF16, tag="wo_sb")
            wor = d_wo.rearrange("(h p) o -> h p o", p=128)
            for h in range(HL):
                nc.sync.dma_start(wo_sb[:, h, :], wor[h])

            # ---------- P1: projections + conv + silu ----------
            # conv outputs, channel-major [chan, t]; q pre-scaled by SCALE
            f1T = pp.tile([128, T], BF16, tag="f1T")
            g1T = pp.tile([128, T], BF16, tag="g1T")
            qT = pp.tile([128, HL, T], BF16, tag="qT")
            kT = pp.tile([128, HL, T], BF16, tag="kT")
            vT = pp.tile([128, HL, T], BF16, tag="vT")
            wT = pp.tile([128, HL, T], BF16, tag="wT")

            def project(ct, out_ap):
                """returns 2 psum tiles [128,512] = (X @ Wbig[:, ct])^T."""
                wct = scr.tile([128, NKT, 128], BF16, tag="wct", bufs=2)
                nc.sync.dma_start(wct[:], d_wbig[ct])
                acc = []
                for tt in range(2):
                    ps = psA.tile([128, 512], F32, tag="pA")
                    for kt in range(NKT):
                        nc.tensor.matmul(
                            ps[:],
                            wct[:, kt, :],
                            xt_sb[:, kt, tt * 512:(tt + 1) * 512],
                            start=(kt == 0), stop=(kt == NKT - 1))
                    acc.append(ps)
                return acc

            def conv_silu(acc, cw_col, out_ap, scale):
                """causal conv (KW taps) + silu via tanh; acc: 2 psum tiles."""
                xpad = scr.tile([128, T + KW - 1], BF16, tag="xpad", bufs=2)
                nc.gpsimd.memset(xpad[:, 0:KW - 1], 0.0)
                for tt in range(2):
                    nc.scalar.copy(
                        xpad[:, KW - 1 + tt * 512: KW - 1 + (tt + 1) * 512],
                        acc[tt][:])
                cacc = scr.tile([128, T], BF16, tag="cacc", bufs=2)
                nc.vector.tensor_scalar_mul(
                    cacc[:], xpad[:, 0:T], c_convw[:, cw_col, 0:1])
                for i in range(1, KW):
                    nc.vector.scalar_tensor_tensor(
                        cacc[:], xpad[:, i:i + T], c_convw[:, cw_col, i:i + 1],
                        cacc[:], op0=ALU.mult, op1=ALU.add)
                # silu = x * 1/(1+e^-x), scaled
                for b in range(2):
                    bs = slice(b * 512, (b + 1) * 512)
                    ep = scr.tile([128, 512], F32, tag="ep", bufs=2)
                    nc.scalar.activation(ep[:], cacc[:, bs], AF.Exp, scale=-1.0)
                    nc.gpsimd.tensor_scalar_add(ep[:], ep[:], 1.0)
                    rp = scr.tile([128, 512], F32, tag="rp", bufs=2)
                    nc.vector.reciprocal_approx_fast(out=rp[:], in_=ep[:])
                    nc.vector.scalar_tensor_tensor(
                        out_ap[:, bs], cacc[:, bs], scale, rp[:],
                        op0=ALU.mult, op1=ALU.mult)

            # order: f1, w, g1, beta first (P2a deps), then k, v, q
            accs = project(8, None)
            for tt in range(2):
                nc.scalar.copy(f1T[:, tt * 512:(tt + 1) * 512], accs[tt][:])
            for h in range(HL):
                conv_silu(project(6 + h, None), 6 + h, wT[:, h, :], 1.0)
            accs = project(9, None)
            for tt in range(2):
                nc.scalar.copy(g1T[:, tt * 512:(tt + 1) * 512], accs[tt][:])
            # beta: [2, T] tiny
            beta_sb = pp.tile([HL, T], BF16, tag="beta_sb")
            for tt in range(2):
                ps = psA.tile([HL, 512], F32, tag="pA")
                for kt in range(NKT):
                    nc.tensor.matmul(
                        ps[:], wb_sb[:, kt, :],
                        xt_sb[:, kt, tt * 512:(tt + 1) * 512],
                        start=(kt == 0), stop=(kt == NKT - 1))
                bth = scr.tile([HL, 512], F32, tag="bth")
                nc.scalar.activation(bth[:], ps[:], AF.Exp, scale=-1.0)
                nc.vector.tensor_scalar_add(bth[:], bth[:], 1.0)
                brc = scr.tile([HL, 512], F32, tag="brc")
                nc.vector.reciprocal_approx_fast(out=brc[:], in_=bth[:])
                nc.vector.tensor_copy(
                    beta_sb[:, tt * 512:(tt + 1) * 512], brc[:])
            for h in range(HL):
                conv_silu(project(2 + h, None), 2 + h, kT[:, h, :], 1.0)
            for h in range(HL):
                conv_silu(project(4 + h, None), 4 + h, vT[:, h, :], 1.0)
            for h in range(HL):
                conv_silu(project(0 + h, None), 0 + h, qT[:, h, :], SCALE)

            # ---------- P2a: gate logits + l2norm scalars (batched Ln) ----------
            gpos_all = pp.tile([C, NCH, HL * M], F32, tag="gpos_all")
            wps_all = pp.tile([C, NCH, HL * M], BF16, tag="wps_all")
            beta_c = pp.tile([C, NCH, HL], F32, tag="beta_c")
            ss_all = pp.tile([C, NCH, HL], F32, tag="ss_all")
            rsb_all = pp.tile([C, NCH, HL], F32, tag="rsb_all")
            for n in range(NCH):
                t0 = n * C
                gps = psB.tile([C, HL * M], F32, tag="pB")
                nc.tensor.matmul(gps[:], f1T[:, t0:t0 + C], c_wf2[:],
                                 start=True, stop=True)
                nc.scalar.activation(gpos_all[:, n, :], gps[:], AF.Exp,
                                     scale=-1.0)
                # w transposes + beta transpose
                for h in range(HL):
                    tp = psC.tile([C, 128], BF16, tag="pC")
                    nc.tensor.transpose(tp[:], wT[:, h, t0:t0 + C], c_identb[:])
                    nc.scalar.copy(wps_all[:, n, h * M:(h + 1) * M], tp[:])
                bt = psC.tile([C, HL], BF16, tag="pC")
                nc.tensor.transpose(bt[:], beta_sb[:, t0:t0 + C],
                                    c_identb[0:HL, 0:HL])
                nc.scalar.copy(beta_c[:, n, :], bt[:])
            for n in range(NCH):
                for h in range(HL):
                    w2 = scr.tile([C, M], BF16, tag="w2")
                    nc.scalar.activation(
                        w2[:], wps_all[:, n, h * M:(h + 1) * M], AF.Square,
                        accum_out=ss_all[:, n, h:h + 1])
            # --- Ln region (single act-table switch) ---
            for n in range(NCH):
                nc.scalar.activation(gpos_all[:, n, :], gpos_all[:, n, :],
                                     AF.Ln, bias=1.0)
            sd_all = pp.tile([C, NCH, HL], F32, tag="sd_all")
            for n in range(NCH):
                for h in range(HL):
                    nc.scalar.activation(sd_all[:, n, h:h + 1],
                                         ss_all[:, n, h:h + 1],
                                         AF.Ln, bias=c_eps6[:])
            # --- back to exp table ---
            for n in range(NCH):
                for h in range(HL):
                    rs = scr.tile([C, 1], F32, tag="rs")
                    nc.scalar.activation(rs[:], sd_all[:, n, h:h + 1],
                                         AF.Exp, scale=-0.5)
                    nc.vector.tensor_mul(rsb_all[:, n, h:h + 1], rs[:],
                                         beta_c[:, n, h:h + 1])

            # ---------- P2b: per-chunk decay tensors + intra scores ----------
            Lam_all = pp.tile([C, NCH, HL * M], BF16, tag="Lam_all")
            Epos_all = pp.tile([C, NCH, HL * M], BF16, tag="Epos_all")
            Eneg_all = pp.tile([C, NCH, HL * M], BF16, tag="Eneg_all")
            EnegT_all = pp.tile([M, NCH, HL * C], BF16, tag="EnegT_all")
            Kdec_all = pp.tile([C, NCH, HL * M], BF16, tag="Kdec_all")
            LamCb_all = pp.tile([128, NCH, HL * M], BF16, tag="LamCb_all")
            LamCc_all = pp.tile([M, NCH, HL], F32, tag="LamCc_all")
            Kc_all = pp.tile([C, NCH, HL * DK], BF16, tag="Kc_all")
            Vc_all = pp.tile([C, NCH, HL * DV], BF16, tag="Vc_all")
            s2_all = pp.tile([C, NCH, HL * M], BF16, tag="s2_all")

            for n in range(NCH):
                t0 = n * C
                gsl = gpos_all[:, n, :]
                pG = psB.tile([C, 2, HL * M], F32, tag="pB")
                nc.tensor.matmul(pG[:, 0, :], c_mcum[:], gsl, start=True, stop=True)
                nc.tensor.matmul(pG[:, 1, :], c_mcen[:], gsl, start=True, stop=True)
                pG2 = psB.tile([C, 2, HL * M], F32, tag="pB")
                nc.tensor.matmul(pG2[:, 0, :], c_mrev[:], gsl, start=True, stop=True)
                nc.tensor.matmul(pG2[:, 1, :], c_negones[:], gsl, start=True, stop=True)
                nc.scalar.activation(Lam_all[:, n, :], pG[:, 0, :], AF.Exp)
                nc.scalar.activation(Epos_all[:, n, :], pG[:, 1, :], AF.Exp)
                enege = scr.tile([C, HL * M], BF16, tag="enege")
                nc.scalar.activation(enege[:], pG[:, 1, :], AF.Exp, scale=-1.0)
                ereve = scr.tile([C, HL * M], BF16, tag="ereve")
                nc.scalar.activation(ereve[:], pG2[:, 0, :], AF.Exp)
                nc.scalar.activation(LamCb_all[:, n, :], pG2[:, 1, :], AF.Exp)
                for h in range(HL):
                    pLcc = psC.tile([M, 1], F32, tag="pC")
                    nc.tensor.matmul(pLcc[:], gsl[:, h * M:(h + 1) * M],
                                     c_negcol[:], start=True, stop=True)
                    nc.scalar.activation(LamCc_all[:, n, h:h + 1], pLcc[:],
                                         AF.Exp)
                # bw, Eneg, Kdec
                bw = scr.tile([C, HL * M], F32, tag="bw")
                for h in range(HL):
                    nc.vector.tensor_scalar_mul(
                        bw[:, h * M:(h + 1) * M],
                        wps_all[:, n, h * M:(h + 1) * M],
                        rsb_all[:, n, h:h + 1])
                nc.vector.tensor_mul(Eneg_all[:, n, :], enege[:], bw[:])
                nc.vector.tensor_mul(Kdec_all[:, n, :], ereve[:], bw[:])
                # K/V/EnegT transposes
                for h in range(HL):
                    tp = psC.tile([C, 128], BF16, tag="pC")
                    nc.tensor.transpose(tp[:], kT[:, h, t0:t0 + C], c_identb[:])
                    nc.vector.tensor_copy(Kc_all[:, n, h * DK:(h + 1) * DK], tp[:])
                    tp2 = psC.tile([C, 128], BF16, tag="pC")
                    nc.tensor.transpose(tp2[:], vT[:, h, t0:t0 + C], c_identb[:])
                    nc.vector.tensor_copy(Vc_all[:, n, h * DV:(h + 1) * DV], tp2[:])
                    tp3 = psC.tile([M, C], BF16, tag="pC")
                    nc.tensor.transpose(
                        tp3[:],
                        Eneg_all[:, n, h * M:(h + 1) * M], c_identb[:])
                    nc.vector.tensor_copy(EnegT_all[:, n, h * C:(h + 1) * C], tp3[:])
                # pt + mask + intra + s2
                for h in range(HL):
                    ppt = psC.tile([C, C], F32, tag="pC")
                    nc.tensor.matmul(ppt[:], kT[:, h, t0:t0 + C],
                                     qT[:, h, t0:t0 + C], start=True, stop=True)
                    Ptm = scr.tile([C, C], BF16, tag="Ptm")
                    nc.vector.tensor_mul(Ptm[:], ppt[:], c_trimask[:])
                    pin = psC.tile([C, M], F32, tag="pC")
                    nc.tensor.matmul(pin[:], Ptm[:],
                                     Eneg_all[:, n, h * M:(h + 1) * M],
                                     start=True, stop=True)
                    nc.vector.tensor_mul(
                        s2_all[:, n, h * M:(h + 1) * M], pin[:],
                        Epos_all[:, n, h * M:(h + 1) * M])

            # ---------- P3: sequential scan core ----------
            Sk = [pp.tile([DK, M], BF16, name=f"Sk{h}", tag=f"Sk{h}") for h in range(HL)]
            Sv = [pp.tile([M, DV], BF16, name=f"Sv{h}", tag=f"Sv{h}") for h in range(HL)]
            for h in range(HL):
                nc.vector.memset(Sk[h][:], 0.0)
                nc.vector.memset(Sv[h][:], 0.0)
            o_pre = pp.tile([C, NCH, HL * DV], F32, tag="o_pre")
            dens = pp.tile([C, NCH, HL], F32, tag="dens")

            for n in range(NCH):
                for h in range(HL):
                    hs = slice(h * M, (h + 1) * M)
                    hc = slice(h * C, (h + 1) * C)
                    t0 = n * C
                    # scores
                    pqs = psC.tile([C, M], F32, tag="pC")
                    nc.tensor.matmul(pqs[:], qT[:, h, t0:t0 + C], Sk[h][:],
                                     start=True, stop=True)
                    sS = scr.tile([C, M], F32, tag="sS")
                    nc.vector.tensor_mul(sS[:], pqs[:], Lam_all[:, n, hs])
                    nc.vector.tensor_add(sS[:], sS[:], s2_all[:, n, hs])
                    pexp = scr.tile([C, M], BF16, tag="pexp")
                    nc.scalar.activation(pexp[:], sS[:], AF.Exp,
                                         accum_out=dens[:, n, h:h + 1])
                    aL = scr.tile([C, M], BF16, tag="aL")
                    nc.gpsimd.tensor_mul(aL[:], pexp[:], Lam_all[:, n, hs])
                    aE = scr.tile([C, M], BF16, tag="aE")
                    nc.gpsimd.tensor_mul(aE[:], pexp[:], Epos_all[:, n, hs])

                    # transposes
                    paLT = psC.tile([M, C], BF16, tag="pC")
                    nc.tensor.transpose(paLT[:], aL[:], c_identb[:])
                    aLT = scr.tile([M, C], BF16, tag="aLT")
                    nc.vector.tensor_copy(aLT[:], paLT[:])
                    paET = psC.tile([M, C], BF16, tag="pC")
                    nc.tensor.transpose(paET[:], aE[:], c_identb[:])
                    aET = scr.tile([M, C], BF16, tag="aET")
                    nc.vector.tensor_copy(aET[:], paET[:])
                    # rt in 2 blocks ([j,i]; skip overflowing j>=HC,i<HC corner)
                    prt = psC.tile([C, C], F32, tag="pC")
                    nc.vector.memset(prt[HC:C, 0:HC], 0.0)
                    nc.tensor.matmul(prt[0:HC, 0:HC],
                                     EnegT_all[:, n, h * C:h * C + HC],
                                     aET[:, 0:HC], start=True, stop=True)
                    nc.tensor.matmul(prt[:, HC:C],
                                     EnegT_all[:, n, hc],
                                     aET[:, HC:C], start=True, stop=True)
                    Rmt = scr.tile([C, C], BF16, tag="Rmt")
                    nc.vector.tensor_mul(Rmt[:], prt[:], c_trimask[:])
                    # output
                    po = psC.tile([C, DV], F32, tag="pC")
                    nc.tensor.matmul(po[:], aLT[:], Sv[h][:],
                                     start=True, stop=False)
                    nc.tensor.matmul(po[:], Rmt[:],
                                     Vc_all[:, n, h * DV:(h + 1) * DV],
                                     start=False, stop=True)
                    nc.vector.tensor_copy(o_pre[:, n, h * DV:(h + 1) * DV], po[:])
                    # state updates
                    pskk = psC.tile([DK, M], F32, tag="pC")
                    nc.tensor.matmul(pskk[:],
                                     Kc_all[:, n, h * DK:(h + 1) * DK],
                                     Kdec_all[:, n, hs], start=True, stop=True)
                    skt = scr.tile([DK, M], BF16, tag="skt")
                    nc.gpsimd.tensor_mul(skt[:], Sk[h][:], LamCb_all[:, n, hs])
                    nc.vector.tensor_add(Sk[h][:], skt[:], pskk[:])
                    psvk = psC.tile([M, DV], F32, tag="pC")
                    nc.tensor.matmul(psvk[:], Kdec_all[:, n, hs],
                                     Vc_all[:, n, h * DV:(h + 1) * DV],
                                     start=True, stop=True)
                    svt = scr.tile([M, DV], BF16, tag="svt")
                    nc.gpsimd.tensor_scalar_mul(svt[:], Sv[h][:],
                                                LamCc_all[:, n, h:h + 1])
                    nc.vector.tensor_add(Sv[h][:], svt[:], psvk[:])

            # ---------- P4: gate + RMSNorm epilogue (batched Ln) ----------
            oT = pp.tile([128, HL, T], BF16, tag="oT")
            oss = pp.tile([C, NCH, HL], F32, tag="oss")
            epsb = pp.tile([C, NCH, HL], F32, tag="epsb")
            om_all = pp.tile([C, NCH, HL], F32, tag="om_all")
            sg_list = []
            for n in range(NCH):
                t0 = n * C
                pgt = psB.tile([C, HL * DV], F32, tag="pB")
                nc.tensor.matmul(pgt[:], g1T[:, t0:t0 + C], c_wg2[:],
                                 start=True, stop=False)
                nc.tensor.matmul(pgt[:], c_ones1[:], c_bg2[:],
                                 start=False, stop=True)
                gth = scr.tile([C, HL * DV], F32, tag="gth", bufs=2)
                nc.scalar.activation(gth[:], pgt[:], AF.Exp, scale=-1.0)
                nc.gpsimd.tensor_scalar_add(gth[:], gth[:], 1.0)
                sg = scr.tile([C, HL * DV], F32, tag="sgate", bufs=8)
                nc.vector.reciprocal_approx_fast(out=sg[:], in_=gth[:])
                sg_list.append(sg)
                for h in range(HL):
                    o2 = scr.tile([C, DV], BF16, tag="o2")
                    nc.scalar.activation(
                        o2[:], o_pre[:, n, h * DV:(h + 1) * DV], AF.Square,
                        accum_out=oss[:, n, h:h + 1])
                    nc.vector.scalar_tensor_tensor(
                        epsb[:, n, h:h + 1], dens[:, n, h:h + 1], EPS,
                        dens[:, n, h:h + 1], op0=ALU.mult, op1=ALU.mult)
            # Ln region
            for n in range(NCH):
                for h in range(HL):
                    nc.scalar.activation(om_all[:, n, h:h + 1],
                                         oss[:, n, h:h + 1], AF.Ln,
                                         scale=1.0 / DV,
                                         bias=epsb[:, n, h:h + 1])
            # back to exp table; finish + transpose
            for n in range(NCH):
                t0 = n * C
                for h in range(HL):
                    rmsf = scr.tile([C, 1], F32, tag="rmsf")
                    nc.scalar.activation(rmsf[:], om_all[:, n, h:h + 1],
                                         AF.Exp, scale=-0.5)
                    of = scr.tile([C, DV], BF16, tag="of")
                    nc.vector.scalar_tensor_tensor(
                        of[:], o_pre[:, n, h * DV:(h + 1) * DV], rmsf[:],
                        sg_list[n][:, h * DV:(h + 1) * DV],
                        op0=ALU.mult, op1=ALU.mult)
                    pot = psC.tile([DV, C], BF16, tag="pC")
                    nc.tensor.transpose(pot[:], of[:], c_identb[:])
                    nc.scalar.copy(oT[:, h, t0:t0 + C], pot[:])

            # ---------- P5: output projection ----------
            for tt in range(8):
                for cl in range(4):
                    ps = psA.tile([128, 512], F32, tag="pA")
                    for h in range(HL):
                        nc.tensor.matmul(
                            ps[:],
                            oT[:, h, tt * 128:(tt + 1) * 128],
                            wo_sb[:, h, cl * 512:(cl + 1) * 512],
                            start=(h == 0), stop=(h == HL - 1))
                    osb = scr.tile([128, 512], FP16, tag="outsb", bufs=3)
                    nc.vector.tensor_copy(osb[:], ps[:])
                    nc.sync.dma_start(
                        d_out[tt * 128:(tt + 1) * 128, cl * 512:(cl + 1) * 512],
                        osb[:])
    nc.compile()
    return nc


def _host_inputs(inputs):
    f32 = np.float32
    bf16 = ml_dtypes.bfloat16
    X = np.ascontiguousarray(np.asarray(inputs["hidden_states"], f32)[0])
    # partition-major X^T: [128, k*T], element (p, k*T+t) = X[t, k*128+p]
    XT = np.ascontiguousarray(
        X.reshape(T, NKT, 128).transpose(2, 1, 0).reshape(128, NKT * T)
    ).astype(bf16)

    j = np.arange(C)[:, None]
    i = np.arange(C)[None, :]
    mcum = -((j <= i).astype(f32))
    mcen = -((j <= i).astype(f32)) + (j <= HC - 1).astype(f32)
    mrev = -((j > i).astype(f32))
    negones = np.full((C, 128), -1.0, f32)
    negcol = np.full((C, 1), -1.0, f32)
    trimask = (j <= i).astype(f32).astype(bf16)
    identb = np.eye(128, dtype=f32).astype(bf16)
    ones1 = np.ones((1, C), f32).astype(bf16)

    Wo_full = np.asarray(inputs["Wo"], f32) * np.tile(
        np.asarray(inputs["norm_w"], f32), H)[:, None]

    Wq = np.asarray(inputs["Wq"], f32)
    Wk = np.asarray(inputs["Wk"], f32)
    Wv = np.asarray(inputs["Wv"], f32)
    Ww = np.asarray(inputs["Ww"], f32)
    Wf1 = np.asarray(inputs["Wf1"], f32)
    Wg1 = np.asarray(inputs["Wg1"], f32)
    cq = np.asarray(inputs["cq"], f32)
    ck = np.asarray(inputs["ck"], f32)
    cv = np.asarray(inputs["cv"], f32)

    in_maps = []
    for c in range(8):
        hsl = slice(c * HL * 128, (c + 1) * HL * 128)
        bsl = slice(c * HL, (c + 1) * HL)
        wbig = np.concatenate(
            [Wq[:, hsl], Wk[:, hsl], Wv[:, hsl], Ww[:, hsl], Wf1, Wg1],
            axis=1)
        # [NCT, 128, NKT*128]: (ct, p, k*128+c) = wbig[k*128+p, ct*128+c]
        wbig_pm = np.ascontiguousarray(
            wbig.reshape(NKT, 128, NCT, 128).transpose(2, 1, 0, 3)
            .reshape(NCT, 128, NKT * 128))
        wb_sl = np.asarray(inputs["Wb"], f32)[:, bsl]
        wb_pm = np.ascontiguousarray(
            wb_sl.reshape(NKT, 128, HL).transpose(1, 0, 2)
            .reshape(128, NKT * HL))
        # conv weights per ct: order q0 q1 k0 k1 v0 v1 w0 w1
        convw = np.zeros((128, 8, KW), f32)
        for hh in range(HL):
            ch = slice((c * HL + hh) * 128, (c * HL + hh + 1) * 128)
            convw[:, 0 + hh] = cq[ch]
            convw[:, 2 + hh] = ck[ch]
            convw[:, 4 + hh] = cv[ch]
            convw[:, 6 + hh] = cv[ch]   # w uses v's conv (faithful to ref)
        m = {
            "xt": XT,
            "wbig": wbig_pm.astype(bf16),
            "wb": wb_pm.astype(bf16),
            "wf2": np.ascontiguousarray(
                np.asarray(inputs["Wf2"], f32)[:, hsl]).astype(bf16),
            "wg2": np.ascontiguousarray(
                np.asarray(inputs["Wg2"], f32)[:, hsl]).astype(bf16),
            "bg2": np.ascontiguousarray(
                np.asarray(inputs["bg2"], f32)[None, hsl]).astype(bf16),
            "wo": np.ascontiguousarray(Wo_full[hsl]).astype(bf16),
            "convw": convw,
            "mcum": mcum, "mcen": mcen, "mrev": mrev,
            "negones": negones, "negcol": negcol,
            "trimask": trimask, "identb": identb, "ones1": ones1,
        }
        in_maps.append(m)
    return in_maps


def kernel(_trace=False, **inputs):
    if "nc" not in _CACHE:
        _CACHE["nc"] = _build_nc()
    nc = _CACHE["nc"]
    in_maps = _host_inputs(inputs)
    res = run_bass_kernel_spmd(nc, in_maps, core_ids=list(range(8)),
                               trace=_trace)
    _CACHE["last_result"] = res
    out = np.zeros((T, HID), np.float32)
    for r in res.results:
        out += np.asarray(r["out"], np.float32)
    return out.reshape(B, T, HID)


# revision 14
# speedup vs baseline: 1.3655x; 1.3655x over previous
"""GatedSlotAttention2 Trainium2 Bass kernel (v2).

Sharding: 2 heads per core x 8 cores (H=16); host sums the 8 partial
Wo outputs. Chunked scan with C=128, all heavy matmul operands in bf16,
state-independent work hoisted out of the sequential loop, single
act-table discipline (Exp/Tanh/Square/Copy + two batched Ln regions),
softmax denominator folded into the RMSNorm eps term.
"""
import numpy as np
import ml_dtypes

import concourse.bass as bass
import concourse.bacc as bacc_mod
import concourse.mybir as mybir
import concourse.tile as tile
from concourse.bass_utils import run_bass_kernel_spmd

# Prefer the activation table that holds Exp+Ln+Square+Copy together so the
# act-table placement pass never needs a mid-kernel table switch.
_orig_get_tables = bacc_mod.get_activation_tables
def _pinned_tables(arch):
    # act_func_set_id is positional (index into act_info.json), so keep
    # order/names and instead empty the sets before the preferred table so
    # first-match resolves every used func to it.
    tabs = _orig_get_tables(arch)
    pref = 'natural_log_exp_and_others'
    if pref not in tabs:
        return tabs
    out = {}
    seen_pref = False
    for k, v in tabs.items():
        if k == pref:
            seen_pref = True
            out[k] = v
        else:
            out[k] = v if seen_pref else type(v)()
    return out
bacc_mod.get_activation_tables = _pinned_tables

F32 = mybir.dt.float32
BF16 = mybir.dt.bfloat16
FP16 = mybir.dt.float16
AF = mybir.ActivationFunctionType
ALU = mybir.AluOpType
MS = bass.MemorySpace

B, T, HID = 1, 1024, 2048
H, DK, DV, M, KW = 16, 128, 128, 128, 4
SCALE = DK ** -0.5
EPS = 1e-5
C = 128           # chunk length
HC = C // 2
NCH = T // C      # 8 chunks
NKT = HID // 128  # 16 contraction tiles
HL = 2            # heads per core
NCT = 10          # 128-wide projection column tiles in wbig

_CACHE = {}


def _build_nc():
    nc = bacc_mod.Bacc("TRN2")

    # ---------------- DRAM I/O ----------------
    d_xt = nc.dram_tensor("xt", [128, NKT * T], BF16, kind="ExternalInput")
    d_wbig = nc.dram_tensor("wbig", [NCT, 128, NKT * 128], BF16, kind="ExternalInput")
    d_wb = nc.dram_tensor("wb", [128, NKT * HL], BF16, kind="ExternalInput")
    d_wf2 = nc.dram_tensor("wf2", [DV, HL * M], BF16, kind="ExternalInput")
    d_wg2 = nc.dram_tensor("wg2", [DV, HL * DV], BF16, kind="ExternalInput")
    d_bg2 = nc.dram_tensor("bg2", [1, HL * DV], BF16, kind="ExternalInput")
    d_wo = nc.dram_tensor("wo", [HL * DV, HID], BF16, kind="ExternalInput")
    d_convw = nc.dram_tensor("convw", [128, 8, KW], F32, kind="ExternalInput")
    d_mcum = nc.dram_tensor("mcum", [C, C], BF16, kind="ExternalInput")
    d_mcen = nc.dram_tensor("mcen", [C, C], BF16, kind="ExternalInput")
    d_mrev = nc.dram_tensor("mrev", [C, C], BF16, kind="ExternalInput")
    d_negones = nc.dram_tensor("negones", [C, 128], BF16, kind="ExternalInput")
    d_negcol = nc.dram_tensor("negcol", [C, 1], BF16, kind="ExternalInput")
    d_trimask = nc.dram_tensor("trimask", [C, C], BF16, kind="ExternalInput")
    d_identb = nc.dram_tensor("identb", [128, 128], BF16, kind="ExternalInput")
    d_ones1 = nc.dram_tensor("ones1", [1, C], BF16, kind="ExternalInput")
    d_out = nc.dram_tensor("out", [T, HID], FP16, kind="ExternalOutput")

    with tile.TileContext(nc) as tc:
        with (
            tc.tile_pool(name="persist", bufs=1) as pp,
            tc.tile_pool(name="scr", bufs=3) as scr,
            tc.tile_pool(name="psA", bufs=2, space=MS.PSUM) as psA,
            tc.tile_pool(name="psB", bufs=2, space=MS.PSUM) as psB,
            tc.tile_pool(name="psC", bufs=4, space=MS.PSUM) as psC,
        ):
            # ---------- constants ----------
            def load_const(dram, shape, dtype=F32):
                t = pp.tile(shape, dtype, tag=dram.name + "_sb")
                nc.sync.dma_start(t[:], dram[:])
                return t

            c_mcum = load_const(d_mcum, [C, C], BF16)
            c_mcen = load_const(d_mcen, [C, C], BF16)
            c_mrev = load_const(d_mrev, [C, C], BF16)
            c_negones = load_const(d_negones, [C, 128], BF16)
            c_negcol = load_const(d_negcol, [C, 1], BF16)
            c_trimask = load_const(d_trimask, [C, C], BF16)
            c_identb = load_const(d_identb, [128, 128], BF16)
            c_ones1 = load_const(d_ones1, [1, C], BF16)
            c_wf2 = load_const(d_wf2, [DV, HL * M], BF16)
            c_wg2 = load_const(d_wg2, [DV, HL * DV], BF16)
            c_bg2 = load_const(d_bg2, [1, HL * DV], BF16)
            c_convw = load_const(d_convw, [128, 8, KW])
            c_eps6 = pp.tile([C, 1], F32, tag="c_eps6")
            nc.vector.memset(c_eps6[:], 1e-6)

            # ---------- big loads ----------
            xt_sb = pp.tile([128, NKT, T], BF16, tag="xt_sb")
            nc.sync.dma_start(xt_sb[:], d_xt[:])
            wb_sb = pp.tile([128, NKT, HL], BF16, tag="wb_sb")
            nc.sync.dma_start(wb_sb[:], d_wb[:])
            wo_sb = pp.tile([128, HL, HID], BF16, tag="wo_sb")
            wor = d_wo.rearrange("(h p) o -> h p o", p=128)
            for h in range(HL):
                nc.sync.dma_start(wo_sb[:, h, :], wor[h])

            # ---------- P1: projections + conv + silu ----------
            # conv outputs, channel-major [chan, t]; q pre-scaled by SCALE
            f1T = pp.tile([128, T], BF16, tag="f1T")
            g1T = pp.tile([128, T], BF16, tag="g1T")
            qT = pp.tile([128, HL, T], BF16, tag="qT")
            kT = pp.tile([128, HL, T], BF16, tag="kT")
            vT = pp.tile([128, HL, T], BF16, tag="vT")
            wT = pp.tile([128, HL, T], BF16, tag="wT")

            def project(ct, out_ap):
                """returns 2 psum tiles [128,512] = (X @ Wbig[:, ct])^T."""
                wct = scr.tile([128, NKT, 128], BF16, tag="wct", bufs=2)
                nc.sync.dma_start(wct[:], d_wbig[ct])
                acc = []
                for tt in range(2):
                    ps = psA.tile([128, 512], F32, tag="pA")
                    for kt in range(NKT):
                        nc.tensor.matmul(
                            ps[:],
                            wct[:, kt, :],
                            xt_sb[:, kt, tt * 512:(tt + 1) * 512],
                            start=(kt == 0), stop=(kt == NKT - 1))
                    acc.append(ps)
                return acc

            def conv_silu(acc, cw_col, out_ap, scale):
                """causal conv (KW taps) + silu via tanh; acc: 2 psum tiles."""
                xpad = scr.tile([128, T + KW - 1], BF16, tag="xpad", bufs=2)
                nc.gpsimd.memset(xpad[:, 0:KW - 1], 0.0)
                for tt in range(2):
                    nc.scalar.copy(
                        xpad[:, KW - 1 + tt * 512: KW - 1 + (tt + 1) * 512],
                        acc[tt][:])
                cacc = scr.tile([128, T], BF16, tag="cacc", bufs=2)
                nc.vector.tensor_scalar_mul(
                    cacc[:], xpad[:, 0:T], c_convw[:, cw_col, 0:1])
                for i in range(1, KW):
                    nc.vector.scalar_tensor_tensor(
                        cacc[:], xpad[:, i:i + T], c_convw[:, cw_col, i:i + 1],
                        cacc[:], op0=ALU.mult, op1=ALU.add)
                # silu = x * 1/(1+e^-x), scaled
                for b in range(2):
                    bs = slice(b * 512, (b + 1) * 512)
                    ep = scr.tile([128, 512], F32, tag="ep", bufs=2)
                    nc.scalar.activation(ep[:], cacc[:, bs], AF.Exp, scale=-1.0)
                    nc.vector.tensor_scalar_add(ep[:], ep[:], 1.0)
                    rp = scr.tile([128, 512], F32, tag="rp", bufs=2)
                    nc.vector.reciprocal_approx_fast(out=rp[:], in_=ep[:])
                    nc.vector.scalar_tensor_tensor(
                        out_ap[:, bs], cacc[:, bs], scale, rp[:],
                        op0=ALU.mult, op1=ALU.mult)

            # order: f1, w, g1, beta first (P2a deps), then k, v, q
            accs = project(8, None)
            for tt in range(2):
                nc.scalar.copy(f1T[:, tt * 512:(tt + 1) * 512], accs[tt][:])
            for h in range(HL):
                conv_silu(project(6 + h, None), 6 + h, wT[:, h, :], 1.0)
            accs = project(9, None)
            for tt in range(2):
                nc.scalar.copy(g1T[:, tt * 512:(tt + 1) * 512], accs[tt][:])
            # beta: [2, T] tiny
            beta_sb = pp.tile([HL, T], BF16, tag="beta_sb")
            for tt in range(2):
                ps = psA.tile([HL, 512], F32, tag="pA")
                for kt in range(NKT):
                    nc.tensor.matmul(
                        ps[:], wb_sb[:, kt, :],
                        xt_sb[:, kt, tt * 512:(tt + 1) * 512],
                        start=(kt == 0), stop=(kt == NKT - 1))
                bth = scr.tile([HL, 512], F32, tag="bth")
                nc.scalar.activation(bth[:], ps[:], AF.Exp, scale=-1.0)
                nc.vector.tensor_scalar_add(bth[:], bth[:], 1.0)
                brc = scr.tile([HL, 512], F32, tag="brc")
                nc.vector.reciprocal_approx_fast(out=brc[:], in_=bth[:])
                nc.vector.tensor_copy(
                    beta_sb[:, tt * 512:(tt + 1) * 512], brc[:])
            for h in range(HL):
                conv_silu(project(2 + h, None), 2 + h, kT[:, h, :], 1.0)
            for h in range(HL):
                conv_silu(project(4 + h, None), 4 + h, vT[:, h, :], 1.0)
            for h in range(HL):
                conv_silu(project(0 + h, None), 0 + h, qT[:, h, :], SCALE)

            # ---------- P2: per-chunk precompute pipeline ----------
            gpos_all = pp.tile([C, NCH, HL * M], BF16, tag="gpos_all")
            wps_all = pp.tile([C, NCH, HL * M], BF16, tag="wps_all")
            beta_c = pp.tile([C, NCH, HL], F32, tag="beta_c")
            rsb_all = pp.tile([C, NCH, HL], F32, tag="rsb_all")
            Lam_all = pp.tile([C, NCH, HL * M], BF16, tag="Lam_all")
            Epos_all = pp.tile([C, NCH, HL * M], BF16, tag="Epos_all")
            Eneg_all = pp.tile([C, NCH, HL * M], BF16, tag="Eneg_all")
            EnegT_all = pp.tile([M, NCH, HL * C], BF16, tag="EnegT_all")
            Kdec_all = pp.tile([C, NCH, HL * M], BF16, tag="Kdec_all")
            LamCb_all = pp.tile([128, NCH, HL * M], BF16, tag="LamCb_all")
            LamCc_all = pp.tile([M, NCH, HL], F32, tag="LamCc_all")
            Kc_all = pp.tile([C, NCH, HL * DK], BF16, tag="Kc_all")
            Vc_all = pp.tile([C, NCH, HL * DV], BF16, tag="Vc_all")
            s2_all = pp.tile([C, NCH, HL * M], BF16, tag="s2_all")

            for n in range(NCH):
                t0 = n * C
                # gate logits -> log-gates (bf16)
                gps = psB.tile([C, HL * M], F32, tag="pB")
                nc.tensor.matmul(gps[:], f1T[:, t0:t0 + C], c_wf2[:],
                                 start=True, stop=True)
                eg = scr.tile([C, HL * M], F32, tag="eg")
                nc.scalar.activation(eg[:], gps[:], AF.Exp, scale=-1.0)
                nc.scalar.activation(gpos_all[:, n, :], eg[:], AF.Ln, bias=1.0)
                # w transpose + l2norm + beta
                for h in range(HL):
                    tp = psC.tile([C, 128], BF16, tag="pC")
                    nc.tensor.transpose(tp[:], wT[:, h, t0:t0 + C], c_identb[:])
                    nc.scalar.copy(wps_all[:, n, h * M:(h + 1) * M], tp[:])
                bt = psC.tile([C, HL], BF16, tag="pC")
                nc.tensor.transpose(bt[:], beta_sb[:, t0:t0 + C],
                                    c_identb[0:HL, 0:HL])
                nc.scalar.copy(beta_c[:, n, :], bt[:])
                for h in range(HL):
                    w2 = scr.tile([C, M], BF16, tag="w2")
                    ss = scr.tile([C, 1], F32, tag="ss")
                    nc.scalar.activation(
                        w2[:], wps_all[:, n, h * M:(h + 1) * M], AF.Square,
                        accum_out=ss[:])
                    sd = scr.tile([C, 1], F32, tag="sd")
                    nc.scalar.activation(sd[:], ss[:], AF.Ln, bias=c_eps6[:])
                    rs = scr.tile([C, 1], F32, tag="rs")
                    nc.scalar.activation(rs[:], sd[:], AF.Exp, scale=-0.5)
                    nc.vector.tensor_mul(rsb_all[:, n, h:h + 1], rs[:],
                                         beta_c[:, n, h:h + 1])
                # cumsums (bf16 moving)
                gsl = gpos_all[:, n, :]
                pG = psB.tile([C, 2, HL * M], F32, tag="pB")
                nc.tensor.matmul(pG[:, 0, :], c_mcum[:], gsl, start=True, stop=True)
                nc.tensor.matmul(pG[:, 1, :], c_mcen[:], gsl, start=True, stop=True)
                pG2 = psB.tile([C, 2, HL * M], F32, tag="pB")
                nc.tensor.matmul(pG2[:, 0, :], c_mrev[:], gsl, start=True, stop=True)
                nc.tensor.matmul(pG2[:, 1, :], c_negones[:], gsl, start=True, stop=True)
                nc.scalar.activation(Lam_all[:, n, :], pG[:, 0, :], AF.Exp)
                nc.scalar.activation(Epos_all[:, n, :], pG[:, 1, :], AF.Exp)
                enege = scr.tile([C, HL * M], BF16, tag="enege")
                nc.scalar.activation(enege[:], pG[:, 1, :], AF.Exp, scale=-1.0)
                ereve = scr.tile([C, HL * M], BF16, tag="ereve")
                nc.scalar.activation(ereve[:], pG2[:, 0, :], AF.Exp)
                nc.scalar.activation(LamCb_all[:, n, :], pG2[:, 1, :], AF.Exp)
                for h in range(HL):
                    pLcc = psC.tile([M, 1], F32, tag="pC")
                    nc.tensor.matmul(pLcc[:], gsl[:, h * M:(h + 1) * M],
                                     c_negcol[:], start=True, stop=True)
                    nc.scalar.activation(LamCc_all[:, n, h:h + 1], pLcc[:],
                                         AF.Exp)
                # bw, Eneg, Kdec
                bw = scr.tile([C, HL * M], F32, tag="bw")
                for h in range(HL):
                    nc.vector.tensor_scalar_mul(
                        bw[:, h * M:(h + 1) * M],
                        wps_all[:, n, h * M:(h + 1) * M],
                        rsb_all[:, n, h:h + 1])
                nc.vector.tensor_mul(Eneg_all[:, n, :], enege[:], bw[:])
                nc.gpsimd.tensor_mul(Kdec_all[:, n, :], ereve[:], bw[:])
                # K/V/EnegT transposes
                for h in range(HL):
                    tp = psC.tile([C, 128], BF16, tag="pC")
                    nc.tensor.transpose(tp[:], kT[:, h, t0:t0 + C], c_identb[:])
                    nc.vector.tensor_copy(Kc_all[:, n, h * DK:(h + 1) * DK], tp[:])
                    tp2 = psC.tile([C, 128], BF16, tag="pC")
                    nc.tensor.transpose(tp2[:], vT[:, h, t0:t0 + C], c_identb[:])
                    nc.vector.tensor_copy(Vc_all[:, n, h * DV:(h + 1) * DV], tp2[:])
                    tp3 = psC.tile([M, C], BF16, tag="pC")
                    nc.tensor.transpose(
                        tp3[:],
                        Eneg_all[:, n, h * M:(h + 1) * M], c_identb[:])
                    nc.vector.tensor_copy(EnegT_all[:, n, h * C:(h + 1) * C], tp3[:])
                # pt + mask + intra + s2
                for h in range(HL):
                    ppt = psC.tile([C, C], F32, tag="pC")
                    nc.tensor.matmul(ppt[:], kT[:, h, t0:t0 + C],
                                     qT[:, h, t0:t0 + C], start=True, stop=True)
                    Ptm = scr.tile([C, C], BF16, tag="Ptm")
                    nc.vector.tensor_mul(Ptm[:], ppt[:], c_trimask[:])
                    pin = psC.tile([C, M], F32, tag="pC")
                    nc.tensor.matmul(pin[:], Ptm[:],
                                     Eneg_all[:, n, h * M:(h + 1) * M],
                                     start=True, stop=True)
                    nc.vector.tensor_mul(
                        s2_all[:, n, h * M:(h + 1) * M], pin[:],
                        Epos_all[:, n, h * M:(h + 1) * M])

            # ---------- P3: sequential scan core ----------
            Sk = [pp.tile([DK, M], BF16, name=f"Sk{h}", tag=f"Sk{h}") for h in range(HL)]
            Sv = [pp.tile([M, DV], BF16, name=f"Sv{h}", tag=f"Sv{h}") for h in range(HL)]
            for h in range(HL):
                nc.vector.memset(Sk[h][:], 0.0)
                nc.vector.memset(Sv[h][:], 0.0)
            o_pre = pp.tile([C, NCH, HL * DV], F32, tag="o_pre")
            dens = pp.tile([C, NCH, HL], F32, tag="dens")

            for n in range(NCH):
                for h in range(HL):
                    hs = slice(h * M, (h + 1) * M)
                    hc = slice(h * C, (h + 1) * C)
                    t0 = n * C
                    # scores
                    pqs = psC.tile([C, M], F32, tag="pC")
                    nc.tensor.matmul(pqs[:], qT[:, h, t0:t0 + C], Sk[h][:],
                                     start=True, stop=True)
                    sS = scr.tile([C, M], F32, tag="sS")
                    nc.vector.tensor_mul(sS[:], pqs[:], Lam_all[:, n, hs])
                    nc.vector.tensor_add(sS[:], sS[:], s2_all[:, n, hs])
                    pexp = scr.tile([C, M], BF16, tag="pexp")
                    nc.scalar.activation(pexp[:], sS[:], AF.Exp,
                                         accum_out=dens[:, n, h:h + 1])
                    aL = scr.tile([C, M], BF16, tag="aL")
                    nc.gpsimd.tensor_mul(aL[:], pexp[:], Lam_all[:, n, hs])
                    aE = scr.tile([C, M], BF16, tag="aE")
                    nc.gpsimd.tensor_mul(aE[:], pexp[:], Epos_all[:, n, hs])

                    # transposes
                    paLT = psC.tile([M, C], BF16, tag="pC")
                    nc.tensor.transpose(paLT[:], aL[:], c_identb[:])
                    aLT = scr.tile([M, C], BF16, tag="aLT")
                    nc.vector.tensor_copy(aLT[:], paLT[:])
                    paET = psC.tile([M, C], BF16, tag="pC")
                    nc.tensor.transpose(paET[:], aE[:], c_identb[:])
                    aET = scr.tile([M, C], BF16, tag="aET")
                    nc.vector.tensor_copy(aET[:], paET[:])
                    # rt in 2 blocks ([j,i]; skip overflowing j>=HC,i<HC corner)
                    prt = psC.tile([C, C], F32, tag="pC")
                    nc.vector.memset(prt[HC:C, 0:HC], 0.0)
                    nc.tensor.matmul(prt[0:HC, 0:HC],
                                     EnegT_all[:, n, h * C:h * C + HC],
                                     aET[:, 0:HC], start=True, stop=True)
                    nc.tensor.matmul(prt[:, HC:C],
                                     EnegT_all[:, n, hc],
                                     aET[:, HC:C], start=True, stop=True)
                    Rmt = scr.tile([C, C], BF16, tag="Rmt")
                    nc.vector.tensor_mul(Rmt[:], prt[:], c_trimask[:])
                    # output
                    po = psC.tile([C, DV], F32, tag="pC")
                    nc.tensor.matmul(po[:], aLT[:], Sv[h][:],
                                     start=True, stop=False)
                    nc.tensor.matmul(po[:], Rmt[:],
                                     Vc_all[:, n, h * DV:(h + 1) * DV],
                                     start=False, stop=True)
                    nc.vector.tensor_copy(o_pre[:, n, h * DV:(h + 1) * DV], po[:])
                    # state updates
                    pskk = psC.tile([DK, M], F32, tag="pC")
                    nc.tensor.matmul(pskk[:],
                                     Kc_all[:, n, h * DK:(h + 1) * DK],
                                     Kdec_all[:, n, hs], start=True, stop=True)
                    skt = scr.tile([DK, M], BF16, tag="skt")
                    nc.gpsimd.tensor_mul(skt[:], Sk[h][:], LamCb_all[:, n, hs])
                    nc.vector.tensor_add(Sk[h][:], skt[:], pskk[:])
                    psvk = psC.tile([M, DV], F32, tag="pC")
                    nc.tensor.matmul(psvk[:], Kdec_all[:, n, hs],
                                     Vc_all[:, n, h * DV:(h + 1) * DV],
                                     start=True, stop=True)
                    svt = scr.tile([M, DV], BF16, tag="svt")
                    nc.gpsimd.tensor_scalar_mul(svt[:], Sv[h][:],
                                                LamCc_all[:, n, h:h + 1])
                    nc.vector.tensor_add(Sv[h][:], svt[:], psvk[:])

            # ---------- P4: gate + RMSNorm epilogue ----------
            oT = pp.tile([128, HL, T], BF16, tag="oT")
            for n in range(NCH):
                t0 = n * C
                pgt = psB.tile([C, HL * DV], F32, tag="pB")
                nc.tensor.matmul(pgt[:], g1T[:, t0:t0 + C], c_wg2[:],
                                 start=True, stop=False)
                nc.tensor.matmul(pgt[:], c_ones1[:], c_bg2[:],
                                 start=False, stop=True)
                gth = scr.tile([C, HL * DV], F32, tag="gth", bufs=2)
                nc.scalar.activation(gth[:], pgt[:], AF.Exp, scale=-1.0)
                nc.vector.tensor_scalar_add(gth[:], gth[:], 1.0)
                sg = scr.tile([C, HL * DV], F32, tag="sgate", bufs=2)
                nc.vector.reciprocal_approx_fast(out=sg[:], in_=gth[:])
                for h in range(HL):
                    o2 = scr.tile([C, DV], BF16, tag="o2")
                    oss = scr.tile([C, 1], F32, tag="oss")
                    nc.scalar.activation(
                        o2[:], o_pre[:, n, h * DV:(h + 1) * DV], AF.Square,
                        accum_out=oss[:])
                    epsb = scr.tile([C, 1], F32, tag="epsb")
                    nc.vector.scalar_tensor_tensor(
                        epsb[:], dens[:, n, h:h + 1], EPS,
                        dens[:, n, h:h + 1], op0=ALU.mult, op1=ALU.mult)
                    om = scr.tile([C, 1], F32, tag="om")
                    nc.scalar.activation(om[:], oss[:], AF.Ln,
                                         scale=1.0 / DV, bias=epsb[:])
                    rmsf = scr.tile([C, 1], F32, tag="rmsf")
                    nc.scalar.activation(rmsf[:], om[:], AF.Exp, scale=-0.5)
                    of = scr.tile([C, DV], BF16, tag="of")
                    nc.vector.scalar_tensor_tensor(
                        of[:], o_pre[:, n, h * DV:(h + 1) * DV], rmsf[:],
                        sg[:, h * DV:(h + 1) * DV],
                        op0=ALU.mult, op1=ALU.mult)
                    pot = psC.tile([DV, C], BF16, tag="pC")
                    nc.tensor.transpose(pot[:], of[:], c_identb[:])
                    nc.scalar.copy(oT[:, h, t0:t0 + C], pot[:])

            # ---------- P5: output projection ----------
            for tt in range(8):
                for cl in range(4):
                    ps = psA.tile([128, 512], F32, tag="pA")
                    for h in range(HL):
                        nc.tensor.matmul(
                            ps[:],
                            oT[:, h, tt * 128:(tt + 1) * 128],
                            wo_sb[:, h, cl * 512:(cl + 1) * 512],
                            start=(h == 0), stop=(h == HL - 1))
                    osb = scr.tile([128, 512], FP16, tag="outsb", bufs=3)
                    nc.vector.tensor_copy(osb[:], ps[:])
                    nc.sync.dma_start(
                        d_out[tt * 128:(tt + 1) * 128, cl * 512:(cl + 1) * 512],
                        osb[:])
    nc.compile()
    return nc


def _host_inputs(inputs):
    f32 = np.float32
    bf16 = ml_dtypes.bfloat16
    X = np.ascontiguousarray(np.asarray(inputs["hidden_states"], f32)[0])
    # partition-major X^T: [128, k*T], element (p, k*T+t) = X[t, k*128+p]
    XT = np.ascontiguousarray(
        X.reshape(T, NKT, 128).transpose(2, 1, 0).reshape(128, NKT * T)
    ).astype(bf16)

    j = np.arange(C)[:, None]
    i = np.arange(C)[None, :]
    mcum = (-((j <= i).astype(f32))).astype(bf16)
    mcen = (-((j <= i).astype(f32)) + (j <= HC - 1).astype(f32)).astype(bf16)
    mrev = (-((j > i).astype(f32))).astype(bf16)
    negones = np.full((C, 128), -1.0, f32).astype(bf16)
    negcol = np.full((C, 1), -1.0, f32).astype(bf16)
    trimask = (j <= i).astype(f32).astype(bf16)
    identb = np.eye(128, dtype=f32).astype(bf16)
    ones1 = np.ones((1, C), f32).astype(bf16)

    Wo_full = np.asarray(inputs["Wo"], f32) * np.tile(
        np.asarray(inputs["norm_w"], f32), H)[:, None]

    Wq = np.asarray(inputs["Wq"], f32)
    Wk = np.asarray(inputs["Wk"], f32)
    Wv = np.asarray(inputs["Wv"], f32)
    Ww = np.asarray(inputs["Ww"], f32)
    Wf1 = np.asarray(inputs["Wf1"], f32)
    Wg1 = np.asarray(inputs["Wg1"], f32)
    cq = np.asarray(inputs["cq"], f32)
    ck = np.asarray(inputs["ck"], f32)
    cv = np.asarray(inputs["cv"], f32)

    in_maps = []
    for c in range(8):
        hsl = slice(c * HL * 128, (c + 1) * HL * 128)
        bsl = slice(c * HL, (c + 1) * HL)
        wbig = np.concatenate(
            [Wq[:, hsl], Wk[:, hsl], Wv[:, hsl], Ww[:, hsl], Wf1, Wg1],
            axis=1)
        # [NCT, 128, NKT*128]: (ct, p, k*128+c) = wbig[k*128+p, ct*128+c]
        wbig_pm = np.ascontiguousarray(
            wbig.reshape(NKT, 128, NCT, 128).transpose(2, 1, 0, 3)
            .reshape(NCT, 128, NKT * 128))
        wb_sl = np.asarray(inputs["Wb"], f32)[:, bsl]
        wb_pm = np.ascontiguousarray(
            wb_sl.reshape(NKT, 128, HL).transpose(1, 0, 2)
            .reshape(128, NKT * HL))
        # conv weights per ct: order q0 q1 k0 k1 v0 v1 w0 w1
        convw = np.zeros((128, 8, KW), f32)
        for hh in range(HL):
            ch = slice((c * HL + hh) * 128, (c * HL + hh + 1) * 128)
            convw[:, 0 + hh] = cq[ch]
            convw[:, 2 + hh] = ck[ch]
            convw[:, 4 + hh] = cv[ch]
            convw[:, 6 + hh] = cv[ch]   # w uses v's conv (faithful to ref)
        m = {
            "xt": XT,
            "wbig": wbig_pm.astype(bf16),
            "wb": wb_pm.astype(bf16),
            "wf2": np.ascontiguousarray(
                np.asarray(inputs["Wf2"], f32)[:, hsl]).astype(bf16),
            "wg2": np.ascontiguousarray(
                np.asarray(inputs["Wg2"], f32)[:, hsl]).astype(bf16),
            "bg2": np.ascontiguousarray(
                np.asarray(inputs["bg2"], f32)[None, hsl]).astype(bf16),
            "wo": np.ascontiguousarray(Wo_full[hsl]).astype(bf16),
            "convw": convw,
            "mcum": mcum, "mcen": mcen, "mrev": mrev,
            "negones": negones, "negcol": negcol,
            "trimask": trimask, "identb": identb, "ones1": ones1,
        }
        in_maps.append(m)
    return in_maps


def kernel(_trace=False, **inputs):
    if "nc" not in _CACHE:
        _CACHE["nc"] = _build_nc()
    nc = _CACHE["nc"]
    in_maps = _host_inputs(inputs)
    res = run_bass_kernel_spmd(nc, in_maps, core_ids=list(range(8)),
                               trace=_trace)
    _CACHE["last_result"] = res
    out = np.zeros((T, HID), np.float32)
    for r in res.results:
        out += np.asarray(r["out"], np.float32)
    return out.reshape(B, T, HID)
